# revision 22
# baseline (speedup 1.0000x reference)
"""Multi-head attention TRN2 Bass kernel, sharded over 8 NeuronCores.

Sharding: core c -> (batch b = c//4, head-group g = c%4).  Each core computes
4 heads' worth of Q/K/V projections + attention for one batch element, plus
the partial output projection for its 256-column slice of the head-concat
dimension.  Host sums the 4 partials per batch and adds bf.

Key tricks (v3):
  - All matmuls fp16 with fp32 PSUM accumulation (fp16 > bf16 precision,
    same PE speed, 2x DVE for 16-bit ops).  fp16 output partials halve the
    output DMA traffic.
  - Host pre-tiles every DRAM tensor so each SBUF tile load is 128
    partition-contiguous runs of 2-8KB (no descriptor storm); tiny constants
    are DMA'd first so nothing latency-critical queues behind megabyte
    weight loads on the HWDGE FIFO ring.
  - Attention is permutation-invariant over keys: the host sorts keys so
    unmasked tokens come first, and the kernel only processes the first
    nkt_a 128-token key chunks.
  - pad_mask folds into the exp as a per-partition log-bias (ln 0 -> -40),
    so masked keys produce es=0 directly; V needs no mask multiply and the
    softmax denominator comes from an all-ones column block in the
    AV-stationary tile.  K-proj padding tokens get lnm=-40 too, so padding
    is automatically ignored.
  - Scores are computed transposed (S^T[kt, qt]); the two heads of a pair use
    disjoint PE row halves (K=64 at base partitions 0/64) and run concurrently.
  - Every matmul pays a serial ~97ns LDWEIGHTS (the bench compiles with
    ldw-opt off), so projection units use N=512 streams and the F stage
    reuses each stationary for both output-column halves.
  - PE warm-up matmuls + a dummy exp (ACT table load) during the initial
    DMA wait.
  - Fine-grained software pipelining: each head-pair's QK/exp loop also
    carries the previous pair's AV accumulation plus split projection /
    output-projection work units (PSUM accumulation groups span interleave
    points), keeping PE dense.
"""

import numpy as np
import ml_dtypes

B, S, DIM, H, DH = 2, 2048, 1024, 16, 64
NCORES = 8
HPC = 4           # heads per core
CSL = HPC * DH    # 256: per-core slice of the head-concat dim
P = 128
KO = DIM // P     # 8 contraction chunks for projections
CC = CSL // P     # 2 col chunks (2 head-pairs)
NKT = S // P      # 16 key-token chunks (full)
QT = 512          # query tile (free dim)
NQT = S // QT     # 4 query tiles
NTOK = S // P     # 16 query-token chunks for y

F16 = np.float16
F8 = ml_dtypes.float8_e4m3

_CACHE = {}
LAST_RESULTS = None


def _build(nkt_a):
    import concourse.bass as bass
    import concourse.tile as tile
    from concourse import bacc, mybir
    from concourse.bass import ts

    f32 = mybir.dt.float32
    f16 = mybir.dt.float16
    f8 = mybir.dt.float8e4
    DR = mybir.MatmulPerfMode.DoubleRow

    KTILES = (nkt_a + 3) // 4          # 512-token K-projection tiles
    KTOK = KTILES * QT                 # padded key-token extent

    nc = bacc.Bacc("TRN2", target_bir_lowering=False, debug=False)

    # Host-pre-tiled layouts: every load is 128 partition-contiguous runs.
    xq = nc.dram_tensor("xq", [NQT, P, KO, QT], f16, kind="ExternalInput").ap()
    xk = nc.dram_tensor("xk", [KTILES, P, KO, QT], f16,
                        kind="ExternalInput").ap()
    xv = nc.dram_tensor("xv", [nkt_a, P, KO, P], f16, kind="ExternalInput").ap()
    wq = nc.dram_tensor("wq", [P, KO, CSL], f16, kind="ExternalInput").ap()
    wk = nc.dram_tensor("wk", [P, KO, CSL], f16, kind="ExternalInput").ap()
    wv = nc.dram_tensor("wv", [P, KO, CSL], f16, kind="ExternalInput").ap()
    wf = nc.dram_tensor("wf", [P, CC, DIM], f16, kind="ExternalInput").ap()
    bq = nc.dram_tensor("bq", [P, CC], f32, kind="ExternalInput").ap()
    bk = nc.dram_tensor("bk", [P, CC], f32, kind="ExternalInput").ap()
    bv = nc.dram_tensor("bv", [CSL], f32, kind="ExternalInput").ap()
    lnm = nc.dram_tensor("lnm", [P, nkt_a], f32, kind="ExternalInput").ap()
    seld = nc.dram_tensor("seld", [4, 2 * P], f32, kind="ExternalInput").ap()
    y = nc.dram_tensor("y", [NTOK, P, DIM], f16, kind="ExternalOutput").ap()

    Exp = mybir.ActivationFunctionType.Exp
    MUL = mybir.AluOpType.mult

    with tile.TileContext(nc) as tc:
        with (
            tc.tile_pool(name="const", bufs=1) as const,
            tc.tile_pool(name="xql", bufs=3) as xql_pool,
            tc.tile_pool(name="xkl", bufs=3) as xkl_pool,
            tc.tile_pool(name="xvl", bufs=3) as xvl_pool,
            tc.tile_pool(name="qkv", bufs=1) as qkv,
            tc.tile_pool(name="es", bufs=3) as es_pool,
            tc.tile_pool(name="ot", bufs=3) as ot_pool,
            tc.tile_pool(name="ysb", bufs=3) as ysb_pool,
            tc.tile_pool(name="rc", bufs=3) as rc_pool,
            tc.tile_pool(name="dscr", bufs=2, space="DRAM") as dram_pool,
            tc.tile_pool(name="stp", bufs=2, space="PSUM") as st_psum,
            tc.tile_pool(name="avp", bufs=2, space="PSUM") as av_psum,
            tc.tile_pool(name="mmp", bufs=2, space="PSUM") as mm_psum,
        ):
            # ---- PE warm-up (HAM) + ACT table pre-load during DMA wait ----
            wu = const.tile([P, P], f16)
            dume = const.tile([P, 8], f16)
            nc.vector.memset(wu, 0.25)
            wps = mm_psum.tile([P, P], f32, tag="mmp", name="warm")
            for i in range(40):
                nc.tensor.matmul(wps, lhsT=wu, rhs=wu,
                                 start=True, stop=True)
            nc.scalar.activation(out=dume, in_=wu[:, 0:8], func=Exp, scale=1.0)

            # ---- constants: tiny ones first, then the lead-path weights ----
            wk_sb = const.tile([P, KO, CSL], f16)
            wq_sb = const.tile([P, KO, CSL], f16)
            wv_sb = const.tile([P, KO, CSL], f16)
            wf_sb = const.tile([P, CC, DIM], f16)
            bk_sb = const.tile([P, CC], f32)
            bq_sb = const.tile([P, CC], f32)
            bv_sb = const.tile([P, CSL], f32)
            lnm_sb = const.tile([P, nkt_a], f32)
            nc.sync.dma_start(bk_sb, bk)
            nc.sync.dma_start(bq_sb, bq)
            nc.sync.dma_start(lnm_sb, lnm)
            nc.sync.dma_start(bv_sb, bv[None, :].to_broadcast((P, CSL)))
            nc.sync.dma_start(wk_sb[:, 0:4, :], wk[:, 0:4, :])
            nc.sync.dma_start(wk_sb[:, 4:8, :], wk[:, 4:8, :])

            sel = const.tile([4, 2, P], f32)
            nc.sync.dma_start(sel, seld.rearrange("r (j p) -> r j p", p=P))

            qt_sb = qkv.tile([P, CC, S], f16)
            kt_sb = qkv.tile([P, CC, KTOK], f16)

            # V in AV-stationary form. Per head h, vaug[:, kc, h, :] is 128
            # wide: even h -> [V(64) | 1s(64)], odd h -> [1s(64) | V(64)].
            # AV psum rows: even: O at 0..63, denom at 64;
            #               odd:  denom at 0, O at 64..127.
            # (es is pre-masked by the exp log-bias, so the denominator
            # column is a constant 1; surplus 1-columns land in psum rows
            # that are never read.)
            vaug = qkv.tile([P, nkt_a, HPC, P], f16)
            vaug_v = vaug.rearrange("p c (hp par) w -> p par c hp w", par=2)
            bv_v = bv_sb.rearrange("p (hp par d) -> p par hp d", par=2, d=DH)

            xq_cache = {}
            xk_cache = {}
            qp_ps = {}
            kp_ps = {}

            def emit_kproj_half(tk, cc, h):
                """Half (4 ko steps) of a (512-token, 128-col) K^T projection
                block; the PSUM accumulation group spans both halves."""
                xt = xk_cache.get(tk)
                if xt is None:
                    xt = xkl_pool.tile([P, KO, QT], f16, tag="xkl",
                                       name=f"xk_{tk}")
                    nc.sync.dma_start(xt[:, 0:4, :], xk[tk][:, 0:4, :])
                    nc.sync.dma_start(xt[:, 4:8, :], xk[tk][:, 4:8, :])
                    xk_cache[tk] = xt
                    if len(xk_cache) > 3:
                        del xk_cache[next(iter(xk_cache))]
                ps = kp_ps.get((tk, cc))
                if ps is None:
                    ps = mm_psum.tile([P, QT], f32, tag="mmp",
                                      name=f"kp{tk}_{cc}")
                    kp_ps[(tk, cc)] = ps
                for ko in range(4 * h, 4 * h + 4):
                    nc.tensor.matmul(
                        ps, lhsT=wk_sb[:, ko, ts(cc, P)], rhs=xt[:, ko, :],
                        start=(ko == 0), stop=(ko == KO - 1),
                    )
                if h == 1:
                    del kp_ps[(tk, cc)]
                    nc.vector.tensor_add(
                        out=kt_sb[:, cc, ts(tk, QT)], in0=ps,
                        in1=bk_sb[:, cc, None].to_broadcast((P, QT)),
                    )

            def emit_qproj_half(t, cc, h):
                """Half (4 ko steps) of a (512-token, 128-col) Q^T projection
                block; the PSUM accumulation group spans both halves."""
                xt = xq_cache.get(t)
                if xt is None:
                    xt = xql_pool.tile([P, KO, QT], f16, tag="xql",
                                       name=f"xq_{t}")
                    nc.sync.dma_start(xt[:, 0:4, :], xq[t][:, 0:4, :])
                    nc.sync.dma_start(xt[:, 4:8, :], xq[t][:, 4:8, :])
                    xq_cache[t] = xt
                    if len(xq_cache) > 3:
                        del xq_cache[next(iter(xq_cache))]
                ps = qp_ps.get((t, cc))
                if ps is None:
                    ps = mm_psum.tile([P, QT], f32, tag="mmp",
                                      name=f"qp{t}_{cc}")
                    qp_ps[(t, cc)] = ps
                for ko in range(4 * h, 4 * h + 4):
                    nc.tensor.matmul(
                        ps, lhsT=wq_sb[:, ko, ts(cc, P)], rhs=xt[:, ko, :],
                        start=(ko == 0), stop=(ko == KO - 1),
                    )
                if h == 1:
                    del qp_ps[(t, cc)]
                    nc.vector.tensor_add(
                        out=qt_sb[:, cc, ts(t, QT)], in0=ps,
                        in1=bq_sb[:, cc, None].to_broadcast((P, QT)),
                    )

            xv_cache = {}

            def load_xv(t):
                if t not in xv_cache and t < nkt_a:
                    xt = xvl_pool.tile([P, KO, P], f16, tag="xvl",
                                       name=f"xv_{t}")
                    nc.sync.dma_start(xt, xv[t])
                    xv_cache[t] = xt

            def emit_vproj_chunk(t):
                """One 128-token chunk of the V projection into vaug.
                Prefetches the chunk two units ahead to hide DMA latency."""
                load_xv(t)
                load_xv(t + 2)
                xt = xv_cache.pop(t)
                ps = mm_psum.tile([P, CSL], f32, tag="mmp", name=f"vp{t}")
                for ko in range(KO):
                    nc.tensor.matmul(
                        ps, lhsT=xt[:, ko, :], rhs=wv_sb[:, ko, :],
                        start=(ko == 0), stop=(ko == KO - 1),
                    )
                ps_v = ps.rearrange("p (hp par d) -> p par hp d", par=2, d=DH)
                for par, dlo in ((0, 0), (1, DH)):
                    nc.vector.tensor_add(
                        out=vaug_v[:, par, t, :, dlo:dlo + DH],
                        in0=ps_v[:, par, :, :], in1=bv_v[:, par, :, :],
                    )

            def emit_f_unit(t, tt):
                """One [128 tok, full 1024 e] block of the output projection.
                The two 512-col halves share each cc stationary (one LDW
                feeds two matmuls)."""
                tok = t * (QT // P) + tt
                psA = mm_psum.tile([P, 512], f32, tag="mmp", name=f"fA{tok}")
                psB = mm_psum.tile([P, 512], f32, tag="mmp", name=f"fB{tok}")
                for cc in range(CC):
                    nc.tensor.matmul(
                        psA, lhsT=ots[t][:, cc, ts(tt, P)],
                        rhs=wf_sb[:, cc, 0:512],
                        start=(cc == 0), stop=(cc == CC - 1),
                    )
                    nc.tensor.matmul(
                        psB, lhsT=ots[t][:, cc, ts(tt, P)],
                        rhs=wf_sb[:, cc, 512:1024],
                        start=(cc == 0), stop=(cc == CC - 1),
                    )
                ysb = ysb_pool.tile([P, 2, 512], f16, tag="ysb",
                                    name=f"ys{tok}")
                nc.vector.tensor_copy(out=ysb[:, 0, :], in_=psA)
                nc.vector.tensor_copy(out=ysb[:, 1, :], in_=psB)
                nc.gpsimd.dma_start(y[tok], ysb)

            class PairState:
                """QK/exp products of one head pair, awaiting AV drain."""

                def __init__(self, t, j):
                    self.t, self.j = t, j
                    self.stp0 = None
                    self.es = es_pool.tile([P, nkt_a, 2, QT], f16, tag="es",
                                           name=f"es{t}_{j}")
                    self.avs = [
                        av_psum.tile([P, QT], f32, tag="avp",
                                     name=f"avp{t}_{j}_{jj}")
                        for jj in range(2)
                    ]
                    self.av_kc = 0
                    self.stg = None

                def av_step(self):
                    kc = self.av_kc
                    for jj in range(2):
                        nc.tensor.matmul(
                            self.avs[jj],
                            lhsT=vaug[:, kc, 2 * self.j + jj, :],
                            rhs=self.es[:, kc, jj, :],
                            start=(kc == 0), stop=(kc == nkt_a - 1),
                        )
                    self.av_kc += 1

                def av_drain(self, upto):
                    while self.av_kc < upto:
                        self.av_step()

                def stage(self):
                    """Copy AV psums to SBUF so the PSUM slots free right
                    after the AV drain instead of after the normalize chain."""
                    t, j = self.t, self.j
                    self.stg = [
                        rc_pool.tile([P, QT], f32, tag="stg", bufs=6,
                                     name=f"sg{t}{j}{jj}")
                        for jj in range(2)
                    ]
                    nc.vector.tensor_copy(
                        out=self.stg[0][0:DH + 1, :], in_=self.avs[0][0:DH + 1, :])
                    nc.vector.tensor_copy(out=self.stg[1], in_=self.avs[1])

            def normalize_t(t, p0, p1):
                """Batched softmax normalization for q-tile t (both pairs).

                Denominator rows live at staged partitions 64 (even head) and
                0 (odd head).  One fast reciprocal for all four rows, then a
                DRAM round-trip to partition-broadcast (only DRAM APs may
                have stride-0 partition dims)."""
                for j, p in ((0, p0), (1, p1)):
                    rcb = rc_pool.tile([P, QT], f32, tag="rcb", name=f"rb{t}{j}")
                    rcr = rc_pool.tile([P, QT], f32, tag="rcr", name=f"rc{t}{j}")
                    nc.gpsimd.partition_broadcast(
                        rcb[0:DH, :], p.stg[0][DH:DH + 1, :])
                    nc.gpsimd.partition_broadcast(
                        rcb[DH:P, :], p.stg[1][0:1, :])
                    nc.vector.reciprocal_approx_fast(out=rcr, in_=rcb)
                    nc.vector.tensor_tensor(
                        out=ots[t][0:DH, j, :], in0=p.stg[0][0:DH, :],
                        in1=rcr[0:DH, :], op=MUL,
                    )
                    nc.vector.tensor_tensor(
                        out=ots[t][DH:P, j, :], in0=p.stg[1][DH:P, :],
                        in1=rcr[DH:P, :], op=MUL,
                    )

            def emit_qk(st, kc):
                stp = st_psum.tile([P, 2, QT], f32, tag="stp",
                                   name=f"st{st.t}_{st.j}_{kc}")
                nc.tensor.matmul(
                    stp[:, 0, :],
                    lhsT=kt_sb[0:DH, st.j, ts(kc, P)],
                    rhs=qt_sb[0:DH, st.j, ts(st.t, QT)],
                    start=True, stop=True,
                )
                nc.tensor.matmul(
                    stp[:, 1, :],
                    lhsT=kt_sb[DH:P, st.j, ts(kc, P)],
                    rhs=qt_sb[DH:P, st.j, ts(st.t, QT)],
                    start=True, stop=True,
                )
                return stp

            def emit_pair(st, units, drain=None, self_av=False,
                          next_st=None):
                """QK+exp loop for pair st, interleaving `units` and the
                AV drain of a previous pair (and optionally its own, lagging
                2 kc behind the exp).  The NEXT pair's first QK is emitted
                ahead of the final drain so its exp starts without a gap."""
                nu = len(units)
                ei = 0
                for kc in range(nkt_a):
                    stp = st.stp0 if kc == 0 and st.stp0 is not None \
                        else emit_qk(st, kc)
                    nc.scalar.activation(
                        out=st.es[:, kc, :, :], in_=stp[:, :, :],
                        func=Exp, scale=1.0 / DH,
                        bias=lnm_sb[:, kc:kc + 1],
                    )
                    if kc == nkt_a - 1 and next_st is not None:
                        next_st.stp0 = emit_qk(next_st, 0)
                    target = (kc + 1) * nu // nkt_a
                    while ei < target:
                        units[ei]()
                        ei += 1
                    if drain is not None:
                        drain.av_drain(kc + 1)
                    if self_av and kc >= 2:
                        st.av_drain(kc - 1)
                if drain is not None:
                    drain.av_drain(nkt_a)
                    drain.stage()
                if self_av:
                    st.av_drain(nkt_a)
                    st.stage()
                return st

            # ---- lead-in: just enough K/Q projection for the first pair ----
            emit_kproj_half(0, 0, 0)
            nc.sync.dma_start(wq_sb[:, 0:4, :], wq[:, 0:4, :])
            nc.sync.dma_start(wq_sb[:, 4:8, :], wq[:, 4:8, :])
            emit_kproj_half(0, 0, 1)
            emit_qproj_half(0, 0, 0)
            emit_qproj_half(0, 0, 1)
            for tk in range(1, KTILES):
                xt = xkl_pool.tile([P, KO, QT], f16, tag="xkl",
                                   name=f"xk_{tk}")
                nc.sync.dma_start(xt, xk[tk])
                xk_cache[tk] = xt
            nc.sync.dma_start(wv_sb, wv)
            nc.sync.dma_start(wf_sb, wf)
            nc.vector.memset(vaug, 1.0)

            ots = {
                t: ot_pool.tile([P, CC, QT], f16, tag="ot", name=f"ot{t}")
                for t in range(NQT)
            }

            # remaining projection blocks as interleavable half-units.
            # cc=0 K tiles first: pair (0,0) consumes kt cc=0 chunks in order.
            k_order = (
                [(tk, 0) for tk in range(1, KTILES)]
                + [(tk, 1) for tk in range(KTILES)]
            )
            k_units = [
                (lambda tk=tk, cc=cc, h=h: emit_kproj_half(tk, cc, h))
                for (tk, cc) in k_order for h in range(2)
            ]
            q0c1 = [
                (lambda h=h: emit_qproj_half(0, 1, h)) for h in range(2)
            ]
            def v_unit0():
                load_xv(0)
                load_xv(1)
                emit_vproj_chunk(0)

            v_units = [v_unit0] + [
                (lambda tt=tt: emit_vproj_chunk(tt))
                for tt in range(1, nkt_a)
            ]

            def prefetch_xq(t):
                def _pf():
                    if t not in xq_cache and t < NQT:
                        xt = xql_pool.tile([P, KO, QT], f16, tag="xql",
                                           name=f"xq_{t}")
                        nc.sync.dma_start(xt[:, 0:4, :], xq[t][:, 0:4, :])
                        nc.sync.dma_start(xt[:, 4:8, :], xq[t][:, 4:8, :])
                        xq_cache[t] = xt
                        if len(xq_cache) > 3:
                            del xq_cache[next(iter(xq_cache))]
                return _pf

            def qproj_units(t):
                return [
                    (lambda cc=cc, h=h, tn=t: emit_qproj_half(tn, cc, h))
                    for cc in range(CC) for h in range(2)
                ]

            def f_units(t):
                return [
                    (lambda tt=tt, tp=t: emit_f_unit(tp, tt))
                    for tt in range(QT // P)
                ]

            # Unit placement: ot(t-1) is complete only at the END of pair
            # (t, 0) (which drains pair (t-1, 1)), so f(t-1) units go in pair
            # (t, 1).  Qproj(t+1) must precede pair (t+1, 0): put it in (t, 0).
            prev = None
            pairs = {}
            states = {}

            def get_state(t, j):
                if (t, j) not in states:
                    states[(t, j)] = PairState(t, j)
                return states[(t, j)]

            for t in range(NQT):
                if t == 0:
                    u0 = k_units + [prefetch_xq(1)] + q0c1 + qproj_units(1)
                    u1 = v_units
                else:
                    u0 = [prefetch_xq(t + 1)] + qproj_units(t + 1) \
                        if t < NQT - 1 else []
                    u1 = f_units(t - 1)
                p0 = get_state(t, 0)
                p1 = get_state(t, 1)
                emit_pair(p0, u0, drain=prev, next_st=p1)
                if t >= 1:
                    normalize_t(t - 1, pairs[t - 1], prev)
                emit_pair(p1, u1, drain=p0, self_av=(t == NQT - 1),
                          next_st=get_state(t + 1, 0) if t < NQT - 1 else None)
                pairs[t] = p0
                prev = p1
            # tail: normalize the last q-tile, then its output projection
            normalize_t(NQT - 1, pairs[NQT - 1], prev)
            for tt in range(QT // P):
                emit_f_unit(NQT - 1, tt)

    nc.compile()
    return nc


def _get_nc(nkt_a):
    if nkt_a not in _CACHE:
        _CACHE[nkt_a] = _build(nkt_a)
    return _CACHE[nkt_a]


def kernel(**inputs):
    global LAST_RESULTS
    query = np.asarray(inputs["query"], np.float32)
    key = np.asarray(inputs["key"], np.float32)
    value = np.asarray(inputs["value"], np.float32)
    pad_mask = np.asarray(inputs["pad_mask"])
    training = int(np.asarray(inputs["training_status"]))
    Wq = np.asarray(inputs["Wq"], np.float32)
    Wk = np.asarray(inputs["Wk"], np.float32)
    Wv = np.asarray(inputs["Wv"], np.float32)
    Wf = np.asarray(inputs["Wf"], np.float32)
    bq = np.asarray(inputs["bq"], np.float32)
    bk = np.asarray(inputs["bk"], np.float32)
    bv = np.asarray(inputs["bv"], np.float32)
    bf = np.asarray(inputs["bf"], np.float32)

    # Per-batch key permutation: unmasked keys first.  Attention is
    # permutation-invariant over keys, and fully-masked key chunks contribute
    # exactly zero (mask folds into the exp as a log-bias), so the kernel
    # only needs ceil(max_unmasked / 128) key chunks.
    m01_full = {}
    perms = {}
    n_act = 1
    for b in range(B):
        if training:
            m = (pad_mask[b, 0, 0, :] != 0).astype(np.float32)
        else:
            m = np.ones(S, np.float32)
        perm = np.argsort(-m, kind="stable")
        m01_full[b] = m[perm]
        perms[b] = perm
        n_act = max(n_act, int(np.ceil(m.sum() / P)))
    nkt_a = min(NKT, max(2, n_act))
    ktok = nkt_a * P
    ktiles = (nkt_a + 3) // 4
    kpad = ktiles * QT

    nc = _get_nc(nkt_a)

    def prep_q(x, b):
        # [S, DIM] -> [NQT, P(dim-part), KO, QT]: (t,p,ko,s) = x[t*QT+s, ko*P+p]
        xt = x[b].T.reshape(KO, P, NQT, QT)          # [ko, p, t, s]
        return np.ascontiguousarray(
            xt.transpose(2, 1, 0, 3)).astype(F16)    # [t, p, ko, s]

    def prep_k(x, b):
        xp = np.zeros((kpad, DIM), np.float32)
        xp[:ktok] = x[b][perms[b]][:ktok]
        xt = xp.T.reshape(KO, P, ktiles, QT)         # [ko, p, tk, s]
        return np.ascontiguousarray(
            xt.transpose(2, 1, 0, 3)).astype(F16)    # [tk, p, ko, s]

    def prep_v(x, b):
        xp = x[b][perms[b]][:ktok]
        xt = xp.T.reshape(KO, P, nkt_a, P)           # [ko, p, c, s]
        return np.ascontiguousarray(
            xt.transpose(2, 1, 0, 3)).astype(F16)    # [c, p, ko, s]

    def prep_w(W, cs, dt=F16):
        ws = W[:, cs].reshape(KO, P, CSL)
        return np.ascontiguousarray(ws.transpose(1, 0, 2)).astype(dt)

    def prep_wf(W, cs):
        ws = W[cs, :].reshape(CC, P, DIM)
        return np.ascontiguousarray(ws.transpose(1, 0, 2)).astype(F16)

    def prep_b(bb, cs):
        return np.ascontiguousarray(bb[cs].reshape(CC, P).T)

    seld = np.zeros((4, 2, P), np.float32)
    seld[0, 0, 0:DH] = 1.0
    seld[1, 0, DH:P] = 1.0
    seld[2, 1, 0:DH] = 1.0
    seld[3, 1, DH:P] = 1.0
    seld = np.ascontiguousarray(seld.reshape(4, 2 * P))

    xT = {}
    lnm_all = {}
    for b in range(B):
        xT[("q", b)] = prep_q(query, b)
        xT[("k", b)] = prep_k(key, b)
        xT[("v", b)] = prep_v(value, b)
        m = np.zeros(nkt_a * P, np.float32)
        n = min(nkt_a * P, S)
        m[:n] = m01_full[b][:n]
        # ln-mask: 0 for keep, -40 for drop (exp(-40) flushes to 0 in f16)
        l = np.where(m > 0, 0.0, -40.0).astype(np.float32)
        lnm_all[b] = np.ascontiguousarray(l.reshape(nkt_a, P).T)  # [P, nkt]

    in_maps = []
    for c in range(NCORES):
        b, g = divmod(c, HPC)
        cs = slice(g * CSL, (g + 1) * CSL)
        in_maps.append({
            "xq": xT[("q", b)],
            "xk": xT[("k", b)],
            "xv": xT[("v", b)],
            "wq": prep_w(Wq, cs),
            "wk": prep_w(Wk, cs),
            "wv": prep_w(Wv, cs),
            "wf": prep_wf(Wf, cs),
            "bq": prep_b(bq, cs),
            "bk": prep_b(bk, cs),
            "bv": np.ascontiguousarray(bv[cs]),
            "lnm": lnm_all[b],
            "seld": seld,
        })

    from concourse.bass_utils import run_bass_kernel_spmd

    res = run_bass_kernel_spmd(nc, in_maps, core_ids=list(range(NCORES)))
    LAST_RESULTS = res

    out = np.zeros((B, S, DIM), np.float32)
    for c in range(NCORES):
        b = c // HPC
        out[b] += res.results[c]["y"].reshape(S, DIM).astype(np.float32)
    out += bf[None, None, :]
    return out
